# revision 22
# baseline (speedup 1.0000x reference)
"""AxialAttention Trainium2 Bass kernel (v2).

Problem: q,k,v of shape (4, 8, 16, 32, 32, 64) = (b, heads, t, h, w, d),
attention along the h axis (axis 3), softmax over keys, out same shape.

512 independent "slabs" (b, heads, t); each slab is w=32 independent
length-32 attention problems with head dim 64.  64 slabs per core,
processed in "quads" of 4 slabs (= 128 partitions), 2 chunks of 16 w.

Key points vs the v1 baseline (284 us -> ~207 us):
  - Host pre-transposes Q,K into the exact SBUF layout the PE wants
    (no on-device DVE transposes) and casts everything to bf16.
  - Scores: one matmul per (s, w) with full K=64 contraction in one
    shot (64-row tile at tile_position (64*(w%2), 32*s)); 2048 score
    MMs/core vs 4096 in v1.  Emission order keeps each (row, col)
    tile in a run: HW hangs if consecutive matmuls drain to the same
    col group from different row tiles (found empirically; sim is
    fine with it).
  - Scores psum is [128=(s,k), 16w, 32q]: exp runs at full 128
    partitions (one ACTIVATE per 16-w chunk instead of per-s [32,512]
    tiles, 4x scalar throughput).
  - PV: one matmul per (s, w) at diagonal tile_position (32s, 32s)
    reading the dense E layout; V in natural layout with a ones
    column -> softmax denominator lands in psum column 64.
  - PV psum tiles are full 2KB banks ([128, 512] f32) addressed
    manually so no matmul output straddles a bank boundary.
  - Device returns unnormalized [*, 65] bf16 (PV | denom); the
    softmax divide happens on host in fp32.
  - Inputs stream on three DMA rings (SP: K, Pool: Q, Act: V),
    output alternates SP/Act by quad; loads prefetch 2 quads ahead
    (io pool bufs=3).
"""

import os
import sys
import numpy as np

for _p in ("/root/.axon_site/_ro/trn_rl_repo", "/opt/trn_rl_repo"):
    if os.path.isdir(_p) and _p not in sys.path:
        sys.path.append(_p)

B, NH, T, H, W, D = 4, 8, 16, 32, 32, 64
N_CORES = 8
NSLAB = B * NH * T  # 512
NSLAB_CORE = NSLAB // N_CORES  # 64
NQUAD = NSLAB_CORE // 4  # 16
NCHUNK = 2  # chunks of 16 w per quad
CW = W // NCHUNK  # 16

_CACHED_NC = None


def _build_nc(n_slabs):
    import concourse.bacc as bacc
    import concourse.mybir as mybir
    from concourse import tile

    dt = mybir.dt
    nq = n_slabs // 4

    nc = bacc.Bacc("TRN2", target_bir_lowering=False, debug=False,
                   num_devices=N_CORES)
    # host layout: x_t[n, p, d, j, i] = X[n, i, 2j+p, d]  (i = h index)
    q_t = nc.dram_tensor("q_t", [n_slabs, 2, D, W // 2, H], dt.bfloat16,
                         kind="ExternalInput").ap()
    k_t = nc.dram_tensor("k_t", [n_slabs, 2, D, W // 2, H], dt.bfloat16,
                         kind="ExternalInput").ap()
    v_in = nc.dram_tensor("v_in", [n_slabs, H, W, D], dt.bfloat16,
                          kind="ExternalInput").ap()
    o_out = nc.dram_tensor("o_out", [n_slabs, H, W, D + 1], dt.bfloat16,
                           kind="ExternalOutput").ap()

    scale = 1.0 / float(np.sqrt(D))

    with tile.TileContext(nc) as tc:
        with tc.tile_pool(name="io", bufs=3) as io_pool, \
             tc.tile_pool(name="oo", bufs=3) as o_pool, \
             tc.tile_pool(name="ee", bufs=3) as e_pool, \
             tc.tile_pool(name="ps_s", bufs=2, space="PSUM") as ps_s, \
             tc.tile_pool(name="ps_v", bufs=2, space="PSUM") as ps_v:

            state = {}

            def emit_loads(g):
                s0 = 4 * g
                KT = io_pool.tile([128, 4, W // 2, H], dt.bfloat16, name="KT")
                QT = io_pool.tile([128, 4, W // 2, H], dt.bfloat16, name="QT")
                V4 = io_pool.tile([128, W, D + 1], dt.bfloat16, name="V4")
                for s_ in range(4):
                    nc.sync.dma_start(
                        out=KT[:, s_, :, :],
                        in_=k_t[s0 + s_].rearrange("p d j i -> (p d) j i"))
                    nc.gpsimd.dma_start(
                        out=QT[:, s_, :, :],
                        in_=q_t[s0 + s_].rearrange("p d j i -> (p d) j i"))
                nc.scalar.dma_start(
                    out=V4[:, :, 0:D],
                    in_=v_in[s0:s0 + 4].rearrange("s h w d -> (s h) w d"))
                nc.vector.memset(V4[:, :, D:D + 1], 1.0)
                out_sb = o_pool.tile([128, W, D + 1], dt.bfloat16,
                                     name="out_sb")
                state[g] = dict(KT=KT, QT=QT, V4=V4, out_sb=out_sb)

            def emit_scores(g, c):
                st = state[g]
                KT, QT = st["KT"], st["QT"]
                psS = ps_s.tile([128, CW, H], dt.float32, name="psS")
                for p in range(2):
                    for s in range(4):
                        for jh in range(CW // 2):
                            jw = 2 * jh + p
                            j = (CW * c + jw) >> 1
                            nc.tensor.matmul(
                                psS[32 * s:32 * s + 32, jw, :],
                                lhsT=KT[64 * p:64 * p + 64, s, j, :],
                                rhs=QT[64 * p:64 * p + 64, s, j, :],
                                start=True, stop=True,
                                tile_position=(64 * p, 32 * s))
                return psS

            def emit_exp(g, c, psS):
                E = e_pool.tile([128, CW, H], dt.bfloat16, name="E")
                nc.scalar.activation(
                    E[:, :, :], psS[:, :, :],
                    mybir.ActivationFunctionType.Exp, scale=scale)
                return E

            def emit_pv(g, c, E):
                st = state[g]
                V4, out_sb = st["V4"], st["out_sb"]
                # full-bank psum tiles (2048B) so matmul outs stay in-bank
                pvA = ps_v.tile([128, 512], dt.float32, name="pvA")
                pvB = ps_v.tile([128, 512], dt.float32, name="pvB")
                pvC = ps_v.tile([128, 512], dt.float32, name="pvC")
                parts = ((pvA, 0, 7), (pvB, 7, 14), (pvC, 14, 16))
                for jw in range(CW):
                    w = CW * c + jw
                    tgt, lo, _ = next(p_ for p_ in parts
                                      if p_[1] <= jw < p_[2])
                    col = (jw - lo) * (D + 1)
                    for s in range(4):
                        nc.tensor.matmul(
                            tgt[32 * s:32 * s + 32, col:col + D + 1],
                            lhsT=E[32 * s:32 * s + 32, jw, :],
                            rhs=V4[32 * s:32 * s + 32, w, :],
                            start=True, stop=True,
                            tile_position=(32 * s, 32 * s))
                w0 = CW * c
                for tgt, lo, hi in parts:
                    n = hi - lo
                    nc.vector.tensor_copy(
                        out=out_sb[:, w0 + lo:w0 + hi, :],
                        in_=tgt[:, 0:n * (D + 1)].rearrange(
                            "p (j e) -> p j e", j=n))

            def emit_finish(g):
                st = state.pop(g)
                s0 = 4 * g
                eng = nc.sync if (g & 1) == 0 else nc.scalar
                eng.dma_start(
                    out=o_out[s0:s0 + 4].rearrange("s h w e -> (s h) w e"),
                    in_=st["out_sb"][:, :, :])

            emit_loads(0)
            if nq > 1:
                emit_loads(1)
            pending = None
            for t in range(NCHUNK * nq):
                g, c = divmod(t, NCHUNK)
                if c == 0 and g + 2 < nq:
                    emit_loads(g + 2)
                psS = emit_scores(g, c)
                if pending is not None:
                    pg, pc, pE = pending
                    emit_pv(pg, pc, pE)
                    if pc == NCHUNK - 1:
                        emit_finish(pg)
                E = emit_exp(g, c, psS)
                pending = (g, c, E)
            pg, pc, pE = pending
            emit_pv(pg, pc, pE)
            emit_finish(pg)
    nc.compile()
    return nc


def _get_nc():
    global _CACHED_NC
    if _CACHED_NC is None:
        _CACHED_NC = _build_nc(NSLAB_CORE)
    return _CACHED_NC


def kernel(q, k, v, decode_step=0, decode_idx=0, _trace=False):
    from concourse.bass_utils import run_bass_kernel_spmd

    import ml_dtypes
    bf16 = ml_dtypes.bfloat16

    def to_t(x):
        # [n, h, w, d] -> [n, p, d, j, i], x_t[n,p,d,j,i] = x[n,i,2j+p,d]
        x = np.asarray(x, dtype=np.float32).reshape(NSLAB, H, W, D)
        x = x.reshape(NSLAB, H, W // 2, 2, D).transpose(0, 3, 4, 2, 1)
        return np.ascontiguousarray(x.astype(bf16))

    qt = to_t(q)
    kt = to_t(k)
    vb = np.ascontiguousarray(
        np.asarray(v, dtype=np.float32).reshape(NSLAB, H, W, D).astype(bf16))

    nc = _get_nc()
    in_maps = []
    for cix in range(N_CORES):
        sl = slice(cix * NSLAB_CORE, (cix + 1) * NSLAB_CORE)
        in_maps.append({
            "q_t": np.ascontiguousarray(qt[sl]),
            "k_t": np.ascontiguousarray(kt[sl]),
            "v_in": np.ascontiguousarray(vb[sl]),
        })
    res = run_bass_kernel_spmd(nc, in_maps, core_ids=list(range(N_CORES)),
                               trace=_trace)
    raw = np.concatenate([r["o_out"] for r in res.results], axis=0)
    raw = raw.astype(np.float32)
    out = raw[..., 0:D] / raw[..., D:D + 1]
    out = out.reshape(B, NH, T, H, W, D)
    if _trace:
        return out, res
    return out


if __name__ == "__main__":
    rng = np.random.default_rng(0)
    shape = (B, NH, T, H, W, D)
    q = rng.standard_normal(shape, dtype=np.float32)
    k = rng.standard_normal(shape, dtype=np.float32)
    v = rng.standard_normal(shape, dtype=np.float32)
    out = kernel(q, k, v)
    print("kernel ran, out shape", out.shape)
